# revision 23
# baseline (speedup 1.0000x reference)
"""Trainium2 Bass kernel: GQA attention with KV cache (decode, Sq=4).

Problem shapes (hardcoded):
  Q [4, 4, 32, 128] f32, K [4, 8192, 8, 128] f32, V [4, 8192, 8, 128] f32,
  cache_seqlens [4] i32 in [4096, 8192].  Output [4, 4, 32, 128] f32.

Sharding: tensor-parallel over the 8 KV heads — core c owns KV head c and
its 4 grouped query heads, for all 4 batches.  Every core therefore does
identical work regardless of cache_seqlens skew.

Design (DMA-bound at ~9.2 MB/core of K+V):
  - K is stored as fp8 e3m4 (x2 scale, clipped to +-15.5); Q is bf16 and
    pre-divided by 2*sqrt(D) so scores come out exact.  V is e3m4 on
    even-numbered 128-position blocks and bf16 on odd ones.  The PE
    accepts mixed-dtype matmuls (fp8 stationary x bf16 moving; verified
    on HW at fp32-level accuracy), so p and Q stay bf16 and the total
    quantization cost is ~1.71e-2 norm rel err vs the 2e-2 gate
    (K-e3m4 1.42e-2, half-V-e3m4 0.96e-2, in quadrature).
  - Per (batch, head) unit, per 128-position block kb of the cache:
      scoresT[s,q]: lhsT = K^T block [d=128, s=128] (fp8, FWL 4x load),
                    rhs  = qt [d=128, q=16] bf16    -> psT [s=128, q=16]
      p = exp(scoresT) via ACT into p_u bf16; host-built 0/1 mask zeroes
      the <=2 tail blocks.
      out^T[dv,q] += lhsT = V block [s=128, dv=128] (natural layout),
                     rhs  = p_u block [s=128, q=16] -> accumulate in PSUM.
      den[1,q]    += lhsT = ones [128,1], rhs = p_u block (PE-side
                     denominator; keeps the DVE off the critical path).
    All matmuls stream only 16 columns; the PE runs ~75 ns/block,
    under the DMA rate.
  - The whole working set (~72 KB/partition) fits in SBUF, so every
    batch gets its own tiles and every DMA is issued up front with no
    buffer-rotation waits.  A single HWDGE queue sustains only ~220-250
    GB/s, so bytes are balanced across both rings (~4.5 MB each).
  - PV runs one 32-block group behind the score stream (software
    pipelining) so the PE never head-of-line blocks on the exp.
  - Finish per batch: bf16 reciprocal of the PE denominator, ones-matmul
    broadcast to [128,16], one DVE mul, store via gpsimd.  Output is
    written as out^T [dv=128, q=16]; the host transposes.
"""

import functools

import numpy as np
import ml_dtypes

import concourse.bacc as bacc
import concourse.mybir as mybir
import concourse.tile as tile
from concourse import bass_utils

B, SQ, H, HKV, D, DV, SMAX = 4, 4, 32, 8, 128, 128, 8192
G = H // HKV  # 4 query heads per KV head
QR = SQ * G  # 16 query rows per (batch, kv-head) unit
BLK = 128  # kv positions per matmul block
GRP = 32  # blocks per PSUM score group
NCORES = 8

F8_DT = mybir.dt.float8e3
F8_NP = np.dtype(ml_dtypes.float8_e3m4)
K_SCALE = 2.0  # K stored as e3m4(2K); Q pre-divided by 2*sqrt(D)
E3M4_MAX = 15.5
BF_DT = mybir.dt.bfloat16
BF_NP = np.dtype(ml_dtypes.bfloat16)
F32 = mybir.dt.float32


def _lean_drain_and_barrier(self, tick_clock, wait_clock):
    """Minimal TileContext exit: a single drain carrying the global-clock
    waits.  The barrier and per-semaphore clears are dropped: each kernel()
    call loads and executes the NEFF exactly once (bass2jax under axon), so
    no later execution observes the dirty semaphores."""
    from concourse.vector_clock import ScopedClock

    drain_inst = self.nc.sync.drain()
    wait_clock.add_sem_waits(
        drain_inst.ins, ScopedClock({None: tick_clock.global_clock})
    )
    popped = self.nc._tile_sem_poison_stack.pop()
    assert popped is self._sem_poison


def _ne(nblk):
    return (nblk + 1) // 2  # even-indexed blocks (e3m4)


def _no(nblk):
    return nblk // 2  # odd-indexed blocks (bf16)


@functools.lru_cache(maxsize=4)
def _build(nblks: tuple[int, ...]):
    """Build + compile the per-core SPMD program for given per-batch block counts."""
    nc = bacc.Bacc("TRN2", target_bir_lowering=False, debug=False)

    qt = nc.dram_tensor("qt", [D, B * QR], BF_DT, kind="ExternalInput")
    kt = [
        nc.dram_tensor(f"kt{b}", [D, n * BLK], F8_DT, kind="ExternalInput")
        for b, n in enumerate(nblks)
    ]
    # V arrives host-swizzled to the SBUF block image ([sl, kb*DV] with
    # v[sl, kb*DV + dv] = V[128*kb + sl, dv]), split by block parity:
    # even blocks e3m4, odd blocks bf16.
    v8 = [
        nc.dram_tensor(f"v8_{b}", [BLK, _ne(n) * DV], F8_DT, kind="ExternalInput")
        for b, n in enumerate(nblks)
    ]
    v16 = [
        nc.dram_tensor(f"v16_{b}", [BLK, _no(n) * DV], BF_DT, kind="ExternalInput")
        for b, n in enumerate(nblks)
    ]
    mask = nc.dram_tensor("mask", [BLK, B * 2 * QR], BF_DT, kind="ExternalInput")
    ones = nc.dram_tensor("ones", [BLK, 1], BF_DT, kind="ExternalInput")
    ones1p = nc.dram_tensor("ones1p", [1, DV], BF_DT, kind="ExternalInput")
    out = nc.dram_tensor("out", [B, DV, QR], F32, kind="ExternalOutput")

    tile.TileContext._drain_and_barrier = _lean_drain_and_barrier
    with tile.TileContext(nc) as tc:
        with (
            tc.tile_pool(name="const", bufs=1) as cpool,
            tc.tile_pool(name="ktp", bufs=4) as ktpool,
            tc.tile_pool(name="vp", bufs=8) as vpool,
            tc.tile_pool(name="pp", bufs=4) as ppool,
            tc.tile_pool(name="small", bufs=4) as spool,
            tc.tile_pool(name="psT", bufs=3, space="PSUM") as psTpool,
            tc.tile_pool(name="psO", bufs=2, space="PSUM") as psOpool,
            tc.tile_pool(name="psDen", bufs=2, space="PSUM") as psDenpool,
            tc.tile_pool(name="psD", bufs=1, space="PSUM") as psDpool,
        ):
            qt_t = cpool.tile([D, B * QR], BF_DT, tag="qt")
            nc.scalar.dma_start(qt_t[:], qt[:])
            ones_t = cpool.tile([BLK, 1], BF_DT, tag="ones")
            nc.gpsimd.dma_start(ones_t[:], ones[:])
            mask_t = cpool.tile([BLK, B * 2 * QR], BF_DT, tag="mask")
            nc.gpsimd.dma_start(mask_t[:], mask[:])
            ones1p_t = cpool.tile([1, DV], BF_DT, tag="ones1p")
            nc.gpsimd.dma_start(ones1p_t[:], ones1p[:])

            # Per-batch group lists: (g0, glen) pairs.
            groups = []
            for b in range(B):
                gl = []
                for g0 in range(0, nblks[b], GRP):
                    gl.append((g0, min(GRP, nblks[b] - g0)))
                groups.append(gl)

            # --- DMA plan: two rings, each fed in global consumption
            # order (K_b before V_b before K_{b+1}), byte-balanced by
            # alternating tensors:
            #   sync:   K0(8+rest), v16_0(16+rest), v8_1, K2, v16_2, v8_3
            #   scalar: qt, v8_0, K1, then (dispensed between exps):
            #           v16_1, v8_2, K3, v16_3
            # The sync ring has no compute, so its plan is all up front;
            # the scalar ring shares its sequencer with the exps, so its
            # later DMAs are interleaved into the compute loop to keep
            # both flowing.
            ktgs = [
                ktpool.tile([D, nblks[b] * BLK], F8_DT, name="ktg", tag="ktg")
                for b in range(B)
            ]
            vg8s = [
                vpool.tile([BLK, _ne(nblks[b]) * DV], F8_DT, name="vg8", tag="vg8")
                for b in range(B)
            ]
            vg16s = [
                vpool.tile([BLK, _no(nblks[b]) * DV], BF_DT, name="vg16", tag="vg16")
                for b in range(B)
            ]
            # sync ring, in consumption order
            s0 = 0
            for nchunk in (8, nblks[0] - 8):
                s1 = s0 + nchunk * BLK
                nc.sync.dma_start(ktgs[0][:, s0:s1], kt[0][:, s0:s1])
                s0 = s1
            h16 = min(16, _no(nblks[0])) * DV
            nc.sync.dma_start(vg16s[0][:, :h16], v16[0][:, :h16])
            nc.sync.dma_start(vg16s[0][:, h16:], v16[0][:, h16:])
            nc.sync.dma_start(vg8s[1][:], v8[1][:])
            nc.sync.dma_start(ktgs[2][:], kt[2][:])
            nc.sync.dma_start(vg16s[2][:], v16[2][:])
            nc.sync.dma_start(vg8s[3][:], v8[3][:])
            # scalar ring: head now, tail dispensed inside the loop
            nc.scalar.dma_start(vg8s[0][:], v8[0][:])
            nc.scalar.dma_start(ktgs[1][:], kt[1][:])
            scalar_tail = [
                lambda: nc.scalar.dma_start(vg16s[1][:], v16[1][:]),
                lambda: nc.scalar.dma_start(vg8s[2][:], v8[2][:]),
                lambda: nc.scalar.dma_start(ktgs[3][:], kt[3][:]),
                lambda: nc.scalar.dma_start(vg16s[3][:], v16[3][:]),
            ]

            # --- compute, PV software-pipelined one group behind ---
            pend = None  # (b, g0, glen)
            p_us = [None] * B
            outps = [None] * B
            denps = [None] * B

            def emit_pv(b, g0, glen):
                nblk = nblks[b]
                for j in range(glen):
                    kb = g0 + j
                    if kb % 2 == 0:
                        vsl = vg8s[b][:, (kb // 2) * DV : (kb // 2 + 1) * DV]
                    else:
                        vsl = vg16s[b][:, (kb // 2) * DV : (kb // 2 + 1) * DV]
                    nc.tensor.matmul(
                        outps[b][:],
                        lhsT=vsl,
                        rhs=p_us[b][:, kb * QR : (kb + 1) * QR],
                        start=(kb == 0),
                        stop=(kb == nblk - 1),
                    )
                # denominator: ones^T @ p accumulates [1, QR] in PSUM.
                # Trivial weight load (1 column); keeps the DVE out of the
                # batch-finish critical path entirely.
                for j in range(glen):
                    kb = g0 + j
                    nc.tensor.matmul(
                        denps[b][:],
                        lhsT=ones_t[:],
                        rhs=p_us[b][:, kb * QR : (kb + 1) * QR],
                        start=(kb == 0),
                        stop=(kb == nblk - 1),
                    )

            def emit_finish(b):
                """Reciprocal + broadcast + scale + store for a finished batch.
                The raw out^T copy runs as soon as the PV chain stops, so
                only recip -> bcast -> mul -> store trail the denominator."""
                out_raw = spool.tile([DV, QR], F32, tag="outraw")
                nc.vector.tensor_copy(out_raw[:], outps[b][:])
                recipT = spool.tile([1, QR], BF_DT, tag="recipT")
                with nc.allow_low_precision(reason="bf16 recip: 0.2% row scale"):
                    nc.vector.reciprocal(recipT[:], denps[b][:])
                recip_bc = psDpool.tile([DV, QR], F32, tag="recipbc")
                nc.tensor.matmul(
                    recip_bc[:], lhsT=ones1p_t[:], rhs=recipT[:], start=True, stop=True
                )
                out_sb = spool.tile([DV, QR], F32, tag="outsb")
                nc.vector.tensor_mul(out_sb[:], out_raw[:], recip_bc[:])
                nc.gpsimd.dma_start(out[b], out_sb[:])

            for b in range(B):
                nblk = nblks[b]
                outps[b] = psOpool.tile([DV, QR], F32, name="outp", tag="outp")
                denps[b] = psDenpool.tile([1, QR], F32, name="denp", tag="denp")
                p_us[b] = ppool.tile([BLK, nblk * QR], BF_DT, name="p_u", tag="p_u")
                ktg = ktgs[b]

                for gi, (g0, glen) in enumerate(groups[b]):
                    # Scores for this group.
                    psT = psTpool.tile([BLK, GRP * QR], F32, tag="psT")  # one 2KB bank
                    for j in range(glen):
                        kb = g0 + j
                        nc.tensor.matmul(
                            psT[:, j * QR : (j + 1) * QR],
                            lhsT=ktg[:, kb * BLK : (kb + 1) * BLK],
                            rhs=qt_t[:, b * QR : (b + 1) * QR],
                            start=True,
                            stop=True,
                        )
                    nc.scalar.activation(
                        p_us[b][:, g0 * QR : (g0 + glen) * QR],
                        psT[:, : glen * QR],
                        mybir.ActivationFunctionType.Exp,
                    )
                    if scalar_tail:
                        scalar_tail.pop(0)()
                    # zero the masked tail (lives in the last two blocks)
                    for i in range(2):
                        kb_m = nblk - 2 + i
                        if g0 <= kb_m < g0 + glen:
                            sl = slice(kb_m * QR, (kb_m + 1) * QR)
                            nc.vector.tensor_mul(
                                p_us[b][:, sl],
                                p_us[b][:, sl],
                                mask_t[:, (b * 2 + i) * QR : (b * 2 + i + 1) * QR],
                            )

                    # PV for the previous group (software pipelining).
                    if pend is not None:
                        pb, pg0, pglen = pend
                        emit_pv(pb, pg0, pglen)
                        if pb != b:
                            emit_finish(pb)
                    pend = (b, g0, glen)

            # drain the pipeline
            pb, pg0, pglen = pend
            emit_pv(pb, pg0, pglen)
            emit_finish(pb)

    nc.compile()
    return nc


def _shard_inputs(Q, K, V, cache_seqlens, nblks):
    """Per-core input maps. Core c owns KV head c (query heads 4c..4c+3)."""
    qs = (np.asarray(Q, dtype=np.float32) / (K_SCALE * np.sqrt(D))).astype(BF_NP)
    K = np.asarray(K, dtype=np.float32)
    V = np.asarray(V, dtype=np.float32)
    cs = np.asarray(cache_seqlens).astype(np.int64)

    ones = np.ones((BLK, 1), np.float32).astype(BF_NP)
    ones1p = np.ones((1, DV), np.float32).astype(BF_NP)

    # 0/1 mask for the last two blocks of each batch: [128, (b, i, q)]
    mask = np.zeros((BLK, B, 2, QR), np.float32)
    sl = np.arange(BLK)
    m_of_r = np.arange(QR) // G
    for b in range(B):
        for i in range(2):
            s = (nblks[b] - 2 + i) * BLK + sl  # absolute kv position
            valid = s[:, None] <= (cs[b] - SQ + m_of_r)[None, :]
            mask[:, b, i, :] = valid.astype(np.float32)
    mask = np.ascontiguousarray(mask.reshape(BLK, B * 2 * QR)).astype(BF_NP)

    in_maps = []
    for c in range(NCORES):
        m = {
            "qt": np.ascontiguousarray(
                qs[:, :, c * G : (c + 1) * G, :].transpose(3, 0, 1, 2)
            ).reshape(D, B * QR),
            "mask": mask,
            "ones": ones,
            "ones1p": ones1p,
        }
        for b in range(B):
            nb = nblks[b]
            sb = nb * BLK
            kc = np.clip(K[b, :sb, c, :].T * K_SCALE, -E3M4_MAX, E3M4_MAX)
            m[f"kt{b}"] = np.ascontiguousarray(kc).astype(F8_NP)
            # swizzle V to the SBUF block image [sl, (kb, dv)], split by
            # block parity: even blocks e3m4, odd blocks bf16
            vb = V[b, :sb, c, :].reshape(nb, BLK, DV)
            ve = vb[0::2].transpose(1, 0, 2).reshape(BLK, _ne(nb) * DV)
            vo = vb[1::2].transpose(1, 0, 2).reshape(BLK, _no(nb) * DV)
            m[f"v8_{b}"] = np.ascontiguousarray(
                np.clip(ve, -E3M4_MAX, E3M4_MAX)
            ).astype(F8_NP)
            m[f"v16_{b}"] = np.ascontiguousarray(vo).astype(BF_NP)
        in_maps.append(m)
    return in_maps


def _run(Q, K, V, cache_seqlens, trace=False, trace_cores=None):
    cs = np.asarray(cache_seqlens).astype(np.int64)
    nblks = tuple(
        int(min((int(cs[b]) + BLK - 1) // BLK, SMAX // BLK)) for b in range(B)
    )
    nc = _build(nblks)
    in_maps = _shard_inputs(Q, K, V, cache_seqlens, nblks)
    res = bass_utils.run_bass_kernel_spmd(
        nc,
        in_maps,
        core_ids=list(range(NCORES)),
        trace=trace,
        trace_cores=trace_cores,
    )
    out = np.empty((B, SQ, H, DV), np.float32)
    for c in range(NCORES):
        for b in range(B):
            # stored as out^T [dv, q]; undo on host
            out[b, :, c * G : (c + 1) * G, :] = (
                res.results[c]["out"][b].T.reshape(SQ, G, DV).astype(np.float32)
            )
    return out, res


def kernel(Q, K, V, cache_seqlens):
    out, _ = _run(Q, K, V, cache_seqlens)
    return out


# revision 24
# speedup vs baseline: 1.0220x; 1.0220x over previous
"""Trainium2 Bass kernel: GQA attention with KV cache (decode, Sq=4).

Problem shapes (hardcoded):
  Q [4, 4, 32, 128] f32, K [4, 8192, 8, 128] f32, V [4, 8192, 8, 128] f32,
  cache_seqlens [4] i32 in [4096, 8192].  Output [4, 4, 32, 128] f32.

Sharding: tensor-parallel over the 8 KV heads — core c owns KV head c and
its 4 grouped query heads, for all 4 batches.  Every core therefore does
identical work regardless of cache_seqlens skew.

Design (DMA-bound at ~9.2 MB/core of K+V):
  - K is stored as fp8 e3m4 (x2 scale, clipped to +-15.5); Q is bf16 and
    pre-divided by 2*sqrt(D) so scores come out exact.  V is e3m4 on
    even-numbered 128-position blocks and bf16 on odd ones.  The PE
    accepts mixed-dtype matmuls (fp8 stationary x bf16 moving; verified
    on HW at fp32-level accuracy), so p and Q stay bf16 and the total
    quantization cost is ~1.71e-2 norm rel err vs the 2e-2 gate
    (K-e3m4 1.42e-2, half-V-e3m4 0.96e-2, in quadrature).
  - Per (batch, head) unit, per 128-position block kb of the cache:
      scoresT[s,q]: lhsT = K^T block [d=128, s=128] (fp8, FWL 4x load),
                    rhs  = qt [d=128, q=16] bf16    -> psT [s=128, q=16]
      p = exp(scoresT) via ACT into p_u bf16; host-built 0/1 mask zeroes
      the <=2 tail blocks.
      out^T[dv,q] += lhsT = V block [s=128, dv=128] (natural layout),
                     rhs  = p_u block [s=128, q=16] -> accumulate in PSUM.
      den[1,q]    += lhsT = ones [128,1], rhs = p_u block (PE-side
                     denominator; keeps the DVE off the critical path).
    All matmuls stream only 16 columns; the PE runs ~75 ns/block,
    under the DMA rate.
  - The whole working set (~72 KB/partition) fits in SBUF, so every
    batch gets its own tiles and every DMA is issued up front with no
    buffer-rotation waits.  A single HWDGE queue sustains only ~220-250
    GB/s, so bytes are balanced across both rings (~4.5 MB each).
  - PV runs one 32-block group behind the score stream (software
    pipelining) so the PE never head-of-line blocks on the exp.
  - Finish per batch: bf16 reciprocal of the PE denominator, ones-matmul
    broadcast to [128,16], one DVE mul, store via gpsimd.  Output is
    written as out^T [dv=128, q=16]; the host transposes.
"""

import functools

import numpy as np
import ml_dtypes

import concourse.bacc as bacc
import concourse.mybir as mybir
import concourse.tile as tile
from concourse import bass_utils

B, SQ, H, HKV, D, DV, SMAX = 4, 4, 32, 8, 128, 128, 8192
G = H // HKV  # 4 query heads per KV head
QR = SQ * G  # 16 query rows per (batch, kv-head) unit
BLK = 128  # kv positions per matmul block
GRP = 32  # blocks per PSUM score group
NCORES = 8

F8_DT = mybir.dt.float8e3
F8_NP = np.dtype(ml_dtypes.float8_e3m4)
K_SCALE = 2.0  # K stored as e3m4(2K); Q pre-divided by 2*sqrt(D)
E3M4_MAX = 15.5
BF_DT = mybir.dt.bfloat16
BF_NP = np.dtype(ml_dtypes.bfloat16)
F32 = mybir.dt.float32


def _lean_drain_and_barrier(self, tick_clock, wait_clock):
    """Minimal TileContext exit: a single drain carrying the global-clock
    waits.  The barrier and per-semaphore clears are dropped: each kernel()
    call loads and executes the NEFF exactly once (bass2jax under axon), so
    no later execution observes the dirty semaphores."""
    from concourse.vector_clock import ScopedClock

    drain_inst = self.nc.sync.drain()
    wait_clock.add_sem_waits(
        drain_inst.ins, ScopedClock({None: tick_clock.global_clock})
    )
    popped = self.nc._tile_sem_poison_stack.pop()
    assert popped is self._sem_poison


def _ne(nblk):
    return (nblk + 1) // 2  # even-indexed blocks (e3m4)


def _no(nblk):
    return nblk // 2  # odd-indexed blocks (bf16)


@functools.lru_cache(maxsize=4)
def _build(nblks: tuple[int, ...]):
    """Build + compile the per-core SPMD program for given per-batch block counts."""
    nc = bacc.Bacc("TRN2", target_bir_lowering=False, debug=False)

    qt = nc.dram_tensor("qt", [D, B * QR], BF_DT, kind="ExternalInput")
    kt = [
        nc.dram_tensor(f"kt{b}", [D, n * BLK], F8_DT, kind="ExternalInput")
        for b, n in enumerate(nblks)
    ]
    # V arrives host-swizzled to the SBUF block image ([sl, kb*DV] with
    # v[sl, kb*DV + dv] = V[128*kb + sl, dv]), split by block parity:
    # even blocks e3m4, odd blocks bf16.
    v8 = [
        nc.dram_tensor(f"v8_{b}", [BLK, _ne(n) * DV], F8_DT, kind="ExternalInput")
        for b, n in enumerate(nblks)
    ]
    v16 = [
        nc.dram_tensor(f"v16_{b}", [BLK, _no(n) * DV], BF_DT, kind="ExternalInput")
        for b, n in enumerate(nblks)
    ]
    mask = nc.dram_tensor("mask", [BLK, B * 2 * QR], BF_DT, kind="ExternalInput")
    ones = nc.dram_tensor("ones", [BLK, 1], BF_DT, kind="ExternalInput")
    ones1p = nc.dram_tensor("ones1p", [1, DV], BF_DT, kind="ExternalInput")
    out = nc.dram_tensor("out", [B, DV, QR], F32, kind="ExternalOutput")

    tile.TileContext._drain_and_barrier = _lean_drain_and_barrier
    with tile.TileContext(nc) as tc:
        with (
            tc.tile_pool(name="const", bufs=1) as cpool,
            tc.tile_pool(name="ktp", bufs=4) as ktpool,
            tc.tile_pool(name="vp", bufs=8) as vpool,
            tc.tile_pool(name="pp", bufs=4) as ppool,
            tc.tile_pool(name="small", bufs=4) as spool,
            tc.tile_pool(name="psT", bufs=3, space="PSUM") as psTpool,
            tc.tile_pool(name="psO", bufs=2, space="PSUM") as psOpool,
            tc.tile_pool(name="psDen", bufs=2, space="PSUM") as psDenpool,
            tc.tile_pool(name="psD", bufs=1, space="PSUM") as psDpool,
        ):
            qt_t = cpool.tile([D, B * QR], BF_DT, tag="qt")
            nc.scalar.dma_start(qt_t[:], qt[:])
            ones_t = cpool.tile([BLK, 1], BF_DT, tag="ones")
            nc.gpsimd.dma_start(ones_t[:], ones[:])
            mask_t = cpool.tile([BLK, B * 2 * QR], BF_DT, tag="mask")
            nc.gpsimd.dma_start(mask_t[:], mask[:])
            ones1p_t = cpool.tile([1, DV], BF_DT, tag="ones1p")
            nc.gpsimd.dma_start(ones1p_t[:], ones1p[:])

            # Per-batch group lists: (g0, glen) pairs.
            groups = []
            for b in range(B):
                gl = []
                for g0 in range(0, nblks[b], GRP):
                    gl.append((g0, min(GRP, nblks[b] - g0)))
                groups.append(gl)

            # --- all DMAs up front, byte-balanced across the two rings ---
            #   sync:   K0(8+rest), K1, K2, K3, v16_b3       (~4.5 MB)
            #   scalar: qt, v8_b0/v16_b0 (split), v8_b1, v16_b1,
            #           v8_b2, v16_b2, v8_b3                 (~4.7 MB)
            ktgs = []
            for b in range(B):
                ktg = ktpool.tile([D, nblks[b] * BLK], F8_DT, name="ktg", tag="ktg")
                if b == 0:
                    s0 = 0
                    for nchunk in (8, nblks[b] - 8):
                        s1 = s0 + nchunk * BLK
                        nc.sync.dma_start(ktg[:, s0:s1], kt[b][:, s0:s1])
                        s0 = s1
                else:
                    nc.sync.dma_start(ktg[:], kt[b][:])
                ktgs.append(ktg)
            vg8s = [
                vpool.tile([BLK, _ne(nblks[b]) * DV], F8_DT, name="vg8", tag="vg8")
                for b in range(B)
            ]
            vg16s = [
                vpool.tile([BLK, _no(nblks[b]) * DV], BF_DT, name="vg16", tag="vg16")
                for b in range(B)
            ]
            # batch 0 split so PV can start after the first 32 blocks land
            h8, h16 = (GRP // 2) * DV, (GRP // 2) * DV
            nc.scalar.dma_start(vg8s[0][:, :h8], v8[0][:, :h8])
            nc.scalar.dma_start(vg16s[0][:, :h16], v16[0][:, :h16])
            nc.scalar.dma_start(vg8s[0][:, h8:], v8[0][:, h8:])
            nc.scalar.dma_start(vg16s[0][:, h16:], v16[0][:, h16:])
            for b in (1, 2):
                nc.scalar.dma_start(vg8s[b][:], v8[b][:])
                nc.scalar.dma_start(vg16s[b][:], v16[b][:])
            nc.scalar.dma_start(vg8s[3][:], v8[3][:])
            nc.sync.dma_start(vg16s[3][:], v16[3][:])

            # --- compute, PV software-pipelined one group behind ---
            pend = None  # (b, g0, glen)
            p_us = [None] * B
            outps = [None] * B
            denps = [None] * B

            def emit_pv(b, g0, glen):
                nblk = nblks[b]
                for j in range(glen):
                    kb = g0 + j
                    if kb % 2 == 0:
                        vsl = vg8s[b][:, (kb // 2) * DV : (kb // 2 + 1) * DV]
                    else:
                        vsl = vg16s[b][:, (kb // 2) * DV : (kb // 2 + 1) * DV]
                    nc.tensor.matmul(
                        outps[b][:],
                        lhsT=vsl,
                        rhs=p_us[b][:, kb * QR : (kb + 1) * QR],
                        start=(kb == 0),
                        stop=(kb == nblk - 1),
                    )
                # denominator: ones^T @ p accumulates [1, QR] in PSUM.
                # Trivial weight load (1 column); keeps the DVE out of the
                # batch-finish critical path entirely.
                for j in range(glen):
                    kb = g0 + j
                    nc.tensor.matmul(
                        denps[b][:],
                        lhsT=ones_t[:],
                        rhs=p_us[b][:, kb * QR : (kb + 1) * QR],
                        start=(kb == 0),
                        stop=(kb == nblk - 1),
                    )

            def emit_finish(b):
                """Reciprocal + broadcast + scale + store for a finished batch.
                The raw out^T copy runs as soon as the PV chain stops, so
                only recip -> bcast -> mul -> store trail the denominator."""
                out_raw = spool.tile([DV, QR], F32, tag="outraw")
                nc.vector.tensor_copy(out_raw[:], outps[b][:])
                recipT = spool.tile([1, QR], BF_DT, tag="recipT")
                with nc.allow_low_precision(reason="bf16 recip: 0.2% row scale"):
                    nc.vector.reciprocal(recipT[:], denps[b][:])
                recip_bc = psDpool.tile([DV, QR], F32, tag="recipbc")
                nc.tensor.matmul(
                    recip_bc[:], lhsT=ones1p_t[:], rhs=recipT[:], start=True, stop=True
                )
                out_sb = spool.tile([DV, QR], F32, tag="outsb")
                nc.vector.tensor_mul(out_sb[:], out_raw[:], recip_bc[:])
                nc.gpsimd.dma_start(out[b], out_sb[:])

            for b in range(B):
                nblk = nblks[b]
                outps[b] = psOpool.tile([DV, QR], F32, name="outp", tag="outp")
                denps[b] = psDenpool.tile([1, QR], F32, name="denp", tag="denp")
                p_us[b] = ppool.tile([BLK, nblk * QR], BF_DT, name="p_u", tag="p_u")
                ktg = ktgs[b]

                for gi, (g0, glen) in enumerate(groups[b]):
                    # Scores for this group.
                    psT = psTpool.tile([BLK, GRP * QR], F32, tag="psT")  # one 2KB bank
                    for j in range(glen):
                        kb = g0 + j
                        nc.tensor.matmul(
                            psT[:, j * QR : (j + 1) * QR],
                            lhsT=ktg[:, kb * BLK : (kb + 1) * BLK],
                            rhs=qt_t[:, b * QR : (b + 1) * QR],
                            start=True,
                            stop=True,
                        )
                    nc.scalar.activation(
                        p_us[b][:, g0 * QR : (g0 + glen) * QR],
                        psT[:, : glen * QR],
                        mybir.ActivationFunctionType.Exp,
                    )
                    # zero the masked tail (lives in the last two blocks)
                    for i in range(2):
                        kb_m = nblk - 2 + i
                        if g0 <= kb_m < g0 + glen:
                            sl = slice(kb_m * QR, (kb_m + 1) * QR)
                            nc.vector.tensor_mul(
                                p_us[b][:, sl],
                                p_us[b][:, sl],
                                mask_t[:, (b * 2 + i) * QR : (b * 2 + i + 1) * QR],
                            )

                    # PV for the previous group (software pipelining).
                    if pend is not None:
                        pb, pg0, pglen = pend
                        emit_pv(pb, pg0, pglen)
                        if pb != b:
                            emit_finish(pb)
                    pend = (b, g0, glen)

            # drain the pipeline
            pb, pg0, pglen = pend
            emit_pv(pb, pg0, pglen)
            emit_finish(pb)

    nc.compile()
    return nc


def _shard_inputs(Q, K, V, cache_seqlens, nblks):
    """Per-core input maps. Core c owns KV head c (query heads 4c..4c+3)."""
    qs = (np.asarray(Q, dtype=np.float32) / (K_SCALE * np.sqrt(D))).astype(BF_NP)
    K = np.asarray(K, dtype=np.float32)
    V = np.asarray(V, dtype=np.float32)
    cs = np.asarray(cache_seqlens).astype(np.int64)

    ones = np.ones((BLK, 1), np.float32).astype(BF_NP)
    ones1p = np.ones((1, DV), np.float32).astype(BF_NP)

    # 0/1 mask for the last two blocks of each batch: [128, (b, i, q)]
    mask = np.zeros((BLK, B, 2, QR), np.float32)
    sl = np.arange(BLK)
    m_of_r = np.arange(QR) // G
    for b in range(B):
        for i in range(2):
            s = (nblks[b] - 2 + i) * BLK + sl  # absolute kv position
            valid = s[:, None] <= (cs[b] - SQ + m_of_r)[None, :]
            mask[:, b, i, :] = valid.astype(np.float32)
    mask = np.ascontiguousarray(mask.reshape(BLK, B * 2 * QR)).astype(BF_NP)

    in_maps = []
    for c in range(NCORES):
        m = {
            "qt": np.ascontiguousarray(
                qs[:, :, c * G : (c + 1) * G, :].transpose(3, 0, 1, 2)
            ).reshape(D, B * QR),
            "mask": mask,
            "ones": ones,
            "ones1p": ones1p,
        }
        for b in range(B):
            nb = nblks[b]
            sb = nb * BLK
            kc = np.clip(K[b, :sb, c, :].T * K_SCALE, -E3M4_MAX, E3M4_MAX)
            m[f"kt{b}"] = np.ascontiguousarray(kc).astype(F8_NP)
            # swizzle V to the SBUF block image [sl, (kb, dv)], split by
            # block parity: even blocks e3m4, odd blocks bf16
            vb = V[b, :sb, c, :].reshape(nb, BLK, DV)
            ve = vb[0::2].transpose(1, 0, 2).reshape(BLK, _ne(nb) * DV)
            vo = vb[1::2].transpose(1, 0, 2).reshape(BLK, _no(nb) * DV)
            m[f"v8_{b}"] = np.ascontiguousarray(
                np.clip(ve, -E3M4_MAX, E3M4_MAX)
            ).astype(F8_NP)
            m[f"v16_{b}"] = np.ascontiguousarray(vo).astype(BF_NP)
        in_maps.append(m)
    return in_maps


def _run(Q, K, V, cache_seqlens, trace=False, trace_cores=None):
    cs = np.asarray(cache_seqlens).astype(np.int64)
    nblks = tuple(
        int(min((int(cs[b]) + BLK - 1) // BLK, SMAX // BLK)) for b in range(B)
    )
    nc = _build(nblks)
    in_maps = _shard_inputs(Q, K, V, cache_seqlens, nblks)
    res = bass_utils.run_bass_kernel_spmd(
        nc,
        in_maps,
        core_ids=list(range(NCORES)),
        trace=trace,
        trace_cores=trace_cores,
    )
    out = np.empty((B, SQ, H, DV), np.float32)
    for c in range(NCORES):
        for b in range(B):
            # stored as out^T [dv, q]; undo on host
            out[b, :, c * G : (c + 1) * G, :] = (
                res.results[c]["out"][b].T.reshape(SQ, G, DV).astype(np.float32)
            )
    return out, res


def kernel(Q, K, V, cache_seqlens):
    out, _ = _run(Q, K, V, cache_seqlens)
    return out


# revision 25
# speedup vs baseline: 1.0240x; 1.0020x over previous
"""Trainium2 Bass kernel: GQA attention with KV cache (decode, Sq=4).

Problem shapes (hardcoded):
  Q [4, 4, 32, 128] f32, K [4, 8192, 8, 128] f32, V [4, 8192, 8, 128] f32,
  cache_seqlens [4] i32 in [4096, 8192].  Output [4, 4, 32, 128] f32.

Sharding: tensor-parallel over the 8 KV heads — core c owns KV head c and
its 4 grouped query heads, for all 4 batches.  Every core therefore does
identical work regardless of cache_seqlens skew.

Design (DMA-bound at ~9.2 MB/core of K+V):
  - K is stored as fp8 e3m4 (x2 scale, clipped to +-15.5); Q is bf16 and
    pre-divided by 2*sqrt(D) so scores come out exact.  V is e3m4 on
    even-numbered 128-position blocks and bf16 on odd ones.  The PE
    accepts mixed-dtype matmuls (fp8 stationary x bf16 moving; verified
    on HW at fp32-level accuracy), so p and Q stay bf16 and the total
    quantization cost is ~1.71e-2 norm rel err vs the 2e-2 gate
    (K-e3m4 1.42e-2, half-V-e3m4 0.96e-2, in quadrature).
  - Per (batch, head) unit, per 128-position block kb of the cache:
      scoresT[s,q]: lhsT = K^T block [d=128, s=128] (fp8, FWL 4x load),
                    rhs  = qt [d=128, q=16] bf16    -> psT [s=128, q=16]
      p = exp(scoresT) via ACT into p_u bf16; host-built 0/1 mask zeroes
      the <=2 tail blocks.
      out^T[dv,q] += lhsT = V block [s=128, dv=128] (natural layout),
                     rhs  = p_u block [s=128, q=16] -> accumulate in PSUM.
      den[1,q]    += lhsT = ones [128,1], rhs = p_u block (PE-side
                     denominator; keeps the DVE off the critical path).
    All matmuls stream only 16 columns; the PE runs ~75 ns/block,
    under the DMA rate.
  - The whole working set (~72 KB/partition) fits in SBUF, so every
    batch gets its own tiles and every DMA is issued up front with no
    buffer-rotation waits.  A single HWDGE queue sustains only ~220-250
    GB/s, so bytes are balanced across both rings (~4.5 MB each).
  - PV runs one 32-block group behind the score stream (software
    pipelining) so the PE never head-of-line blocks on the exp.
  - Finish per batch: bf16 reciprocal of the PE denominator, ones-matmul
    broadcast to [128,16], one DVE mul, store via gpsimd.  Output is
    written as out^T [dv=128, q=16]; the host transposes.
"""

import functools

import numpy as np
import ml_dtypes

import concourse.bacc as bacc
import concourse.mybir as mybir
import concourse.tile as tile
from concourse import bass_utils

B, SQ, H, HKV, D, DV, SMAX = 4, 4, 32, 8, 128, 128, 8192
G = H // HKV  # 4 query heads per KV head
QR = SQ * G  # 16 query rows per (batch, kv-head) unit
BLK = 128  # kv positions per matmul block
GRP = 32  # blocks per PSUM score group
NCORES = 8

F8_DT = mybir.dt.float8e3
F8_NP = np.dtype(ml_dtypes.float8_e3m4)
K_SCALE = 2.0  # K stored as e3m4(2K); Q pre-divided by 2*sqrt(D)
E3M4_MAX = 15.5
BF_DT = mybir.dt.bfloat16
BF_NP = np.dtype(ml_dtypes.bfloat16)
F32 = mybir.dt.float32


def _lean_drain_and_barrier(self, tick_clock, wait_clock):
    """Minimal TileContext exit: a single drain carrying the global-clock
    waits.  The barrier and per-semaphore clears are dropped: each kernel()
    call loads and executes the NEFF exactly once (bass2jax under axon), so
    no later execution observes the dirty semaphores."""
    from concourse.vector_clock import ScopedClock

    drain_inst = self.nc.sync.drain()
    wait_clock.add_sem_waits(
        drain_inst.ins, ScopedClock({None: tick_clock.global_clock})
    )
    popped = self.nc._tile_sem_poison_stack.pop()
    assert popped is self._sem_poison


def _ne(nblk):
    return (nblk + 1) // 2  # even-indexed blocks (e3m4)


def _no(nblk):
    return nblk // 2  # odd-indexed blocks (bf16)


@functools.lru_cache(maxsize=4)
def _build(nblks: tuple[int, ...]):
    """Build + compile the per-core SPMD program for given per-batch block counts."""
    nc = bacc.Bacc("TRN2", target_bir_lowering=False, debug=False)

    qt = nc.dram_tensor("qt", [D, B * QR], BF_DT, kind="ExternalInput")
    kt = [
        nc.dram_tensor(f"kt{b}", [D, n * BLK], F8_DT, kind="ExternalInput")
        for b, n in enumerate(nblks)
    ]
    # V arrives host-swizzled to the SBUF block image, packed per block
    # PAIR as raw bytes: [128 B e3m4 (even block) | 256 B bf16 (odd
    # block)], one uint8 tensor per batch (+ trailing e3m4 block when
    # nblk is odd).  Single large DMAs keep the queue at line rate; the
    # PV matmuls bitcast the slices back to their dtypes.
    def _vxw(n):
        return _no(n) * 384 + (128 if n % 2 else 0)

    vx = [
        nc.dram_tensor(f"vx{b}", [BLK, _vxw(n)], mybir.dt.uint8, kind="ExternalInput")
        for b, n in enumerate(nblks)
    ]
    mask = nc.dram_tensor("mask", [BLK, B * 2 * QR], BF_DT, kind="ExternalInput")
    ones = nc.dram_tensor("ones", [BLK, 1], BF_DT, kind="ExternalInput")
    ones1p = nc.dram_tensor("ones1p", [1, DV], BF_DT, kind="ExternalInput")
    out = nc.dram_tensor("out", [B, DV, QR], F32, kind="ExternalOutput")

    tile.TileContext._drain_and_barrier = _lean_drain_and_barrier
    with tile.TileContext(nc) as tc:
        with (
            tc.tile_pool(name="const", bufs=1) as cpool,
            tc.tile_pool(name="ktp", bufs=4) as ktpool,
            tc.tile_pool(name="vp", bufs=8) as vpool,
            tc.tile_pool(name="pp", bufs=4) as ppool,
            tc.tile_pool(name="small", bufs=4) as spool,
            tc.tile_pool(name="psT", bufs=3, space="PSUM") as psTpool,
            tc.tile_pool(name="psO", bufs=2, space="PSUM") as psOpool,
            tc.tile_pool(name="psDen", bufs=2, space="PSUM") as psDenpool,
            tc.tile_pool(name="psD", bufs=1, space="PSUM") as psDpool,
        ):
            qt_t = cpool.tile([D, B * QR], BF_DT, tag="qt")
            nc.scalar.dma_start(qt_t[:], qt[:])
            ones_t = cpool.tile([BLK, 1], BF_DT, tag="ones")
            nc.gpsimd.dma_start(ones_t[:], ones[:])
            mask_t = cpool.tile([BLK, B * 2 * QR], BF_DT, tag="mask")
            nc.gpsimd.dma_start(mask_t[:], mask[:])
            ones1p_t = cpool.tile([1, DV], BF_DT, tag="ones1p")
            nc.gpsimd.dma_start(ones1p_t[:], ones1p[:])

            # Per-batch group lists: (g0, glen) pairs.
            groups = []
            for b in range(B):
                gl = []
                for g0 in range(0, nblks[b], GRP):
                    gl.append((g0, min(GRP, nblks[b] - g0)))
                groups.append(gl)

            # --- all DMAs up front, byte-balanced across the two rings ---
            #   sync:   K0(8+rest), K1, K2, K3, v16_b3       (~4.5 MB)
            #   scalar: qt, v8_b0/v16_b0 (split), v8_b1, v16_b1,
            #           v8_b2, v16_b2, v8_b3                 (~4.7 MB)
            ktgs = []
            for b in range(B):
                ktg = ktpool.tile([D, nblks[b] * BLK], F8_DT, name="ktg", tag="ktg")
                if b == 0:
                    s0 = 0
                    for nchunk in (8, nblks[b] - 8):
                        s1 = s0 + nchunk * BLK
                        nc.sync.dma_start(ktg[:, s0:s1], kt[b][:, s0:s1])
                        s0 = s1
                else:
                    nc.sync.dma_start(ktg[:], kt[b][:])
                ktgs.append(ktg)
            vxs = [
                vpool.tile([BLK, _vxw(nblks[b])], mybir.dt.uint8, name="vx", tag="vx")
                for b in range(B)
            ]
            # batch 0 split so PV can start after the first 32 blocks land
            hx = (GRP // 2) * 384
            nc.scalar.dma_start(vxs[0][:, :hx], vx[0][:, :hx])
            nc.scalar.dma_start(vxs[0][:, hx:], vx[0][:, hx:])
            for b in (1, 2):
                nc.scalar.dma_start(vxs[b][:], vx[b][:])
            nc.sync.dma_start(vxs[3][:], vx[3][:])

            # --- compute, PV software-pipelined one group behind ---
            pend = None  # (b, g0, glen)
            p_us = [None] * B
            outps = [None] * B
            denps = [None] * B

            def emit_pv(b, g0, glen):
                nblk = nblks[b]
                for j in range(glen):
                    kb = g0 + j
                    o = (kb // 2) * 384
                    if kb % 2 == 0:
                        vsl = vxs[b][:, o : o + 128].bitcast(F8_DT)
                    else:
                        vsl = vxs[b][:, o + 128 : o + 384].bitcast(BF_DT)
                    nc.tensor.matmul(
                        outps[b][:],
                        lhsT=vsl,
                        rhs=p_us[b][:, kb * QR : (kb + 1) * QR],
                        start=(kb == 0),
                        stop=(kb == nblk - 1),
                    )
                # denominator: ones^T @ p accumulates [1, QR] in PSUM.
                # Trivial weight load (1 column); keeps the DVE out of the
                # batch-finish critical path entirely.
                for j in range(glen):
                    kb = g0 + j
                    nc.tensor.matmul(
                        denps[b][:],
                        lhsT=ones_t[:],
                        rhs=p_us[b][:, kb * QR : (kb + 1) * QR],
                        start=(kb == 0),
                        stop=(kb == nblk - 1),
                    )

            def emit_finish(b):
                """Reciprocal + broadcast + scale + store for a finished batch.
                The raw out^T copy runs as soon as the PV chain stops, so
                only recip -> bcast -> mul -> store trail the denominator."""
                out_raw = spool.tile([DV, QR], F32, tag="outraw")
                nc.vector.tensor_copy(out_raw[:], outps[b][:])
                recipT = spool.tile([1, QR], BF_DT, tag="recipT")
                with nc.allow_low_precision(reason="bf16 recip: 0.2% row scale"):
                    nc.vector.reciprocal(recipT[:], denps[b][:])
                recip_bc = psDpool.tile([DV, QR], F32, tag="recipbc")
                nc.tensor.matmul(
                    recip_bc[:], lhsT=ones1p_t[:], rhs=recipT[:], start=True, stop=True
                )
                out_sb = spool.tile([DV, QR], F32, tag="outsb")
                nc.vector.tensor_mul(out_sb[:], out_raw[:], recip_bc[:])
                nc.gpsimd.dma_start(out[b], out_sb[:])

            for b in range(B):
                nblk = nblks[b]
                outps[b] = psOpool.tile([DV, QR], F32, name="outp", tag="outp")
                denps[b] = psDenpool.tile([1, QR], F32, name="denp", tag="denp")
                p_us[b] = ppool.tile([BLK, nblk * QR], BF_DT, name="p_u", tag="p_u")
                ktg = ktgs[b]

                for gi, (g0, glen) in enumerate(groups[b]):
                    # Scores for this group.
                    psT = psTpool.tile([BLK, GRP * QR], F32, tag="psT")  # one 2KB bank
                    for j in range(glen):
                        kb = g0 + j
                        nc.tensor.matmul(
                            psT[:, j * QR : (j + 1) * QR],
                            lhsT=ktg[:, kb * BLK : (kb + 1) * BLK],
                            rhs=qt_t[:, b * QR : (b + 1) * QR],
                            start=True,
                            stop=True,
                        )
                    nc.scalar.activation(
                        p_us[b][:, g0 * QR : (g0 + glen) * QR],
                        psT[:, : glen * QR],
                        mybir.ActivationFunctionType.Exp,
                    )
                    # zero the masked tail (lives in the last two blocks)
                    for i in range(2):
                        kb_m = nblk - 2 + i
                        if g0 <= kb_m < g0 + glen:
                            sl = slice(kb_m * QR, (kb_m + 1) * QR)
                            nc.vector.tensor_mul(
                                p_us[b][:, sl],
                                p_us[b][:, sl],
                                mask_t[:, (b * 2 + i) * QR : (b * 2 + i + 1) * QR],
                            )

                    # PV for the previous group (software pipelining).
                    if pend is not None:
                        pb, pg0, pglen = pend
                        emit_pv(pb, pg0, pglen)
                        if pb != b:
                            emit_finish(pb)
                    pend = (b, g0, glen)

            # drain the pipeline
            pb, pg0, pglen = pend
            emit_pv(pb, pg0, pglen)
            emit_finish(pb)

    nc.compile()
    return nc


def _shard_inputs(Q, K, V, cache_seqlens, nblks):
    """Per-core input maps. Core c owns KV head c (query heads 4c..4c+3)."""
    qs = (np.asarray(Q, dtype=np.float32) / (K_SCALE * np.sqrt(D))).astype(BF_NP)
    K = np.asarray(K, dtype=np.float32)
    V = np.asarray(V, dtype=np.float32)
    cs = np.asarray(cache_seqlens).astype(np.int64)

    ones = np.ones((BLK, 1), np.float32).astype(BF_NP)
    ones1p = np.ones((1, DV), np.float32).astype(BF_NP)

    # 0/1 mask for the last two blocks of each batch: [128, (b, i, q)]
    mask = np.zeros((BLK, B, 2, QR), np.float32)
    sl = np.arange(BLK)
    m_of_r = np.arange(QR) // G
    for b in range(B):
        for i in range(2):
            s = (nblks[b] - 2 + i) * BLK + sl  # absolute kv position
            valid = s[:, None] <= (cs[b] - SQ + m_of_r)[None, :]
            mask[:, b, i, :] = valid.astype(np.float32)
    mask = np.ascontiguousarray(mask.reshape(BLK, B * 2 * QR)).astype(BF_NP)

    in_maps = []
    for c in range(NCORES):
        m = {
            "qt": np.ascontiguousarray(
                qs[:, :, c * G : (c + 1) * G, :].transpose(3, 0, 1, 2)
            ).reshape(D, B * QR),
            "mask": mask,
            "ones": ones,
            "ones1p": ones1p,
        }
        for b in range(B):
            nb = nblks[b]
            sb = nb * BLK
            kc = np.clip(K[b, :sb, c, :].T * K_SCALE, -E3M4_MAX, E3M4_MAX)
            m[f"kt{b}"] = np.ascontiguousarray(kc).astype(F8_NP)
            # swizzle V to the SBUF block image and pack block pairs as
            # raw bytes: [128 B e3m4 even | 256 B bf16 odd]
            vb = V[b, :sb, c, :].reshape(nb, BLK, DV)
            npair = nb // 2
            w = npair * 384 + (128 if nb % 2 else 0)
            arr = np.empty((BLK, w), np.uint8)
            ve = np.clip(vb[0::2], -E3M4_MAX, E3M4_MAX).astype(F8_NP)
            vo = vb[1::2].astype(BF_NP)
            for i in range(npair):
                arr[:, i * 384 : i * 384 + 128] = ve[i].view(np.uint8)
                arr[:, i * 384 + 128 : (i + 1) * 384] = vo[i].view(np.uint8)
            if nb % 2:
                arr[:, npair * 384 :] = ve[npair].view(np.uint8)
            m[f"vx{b}"] = arr
        in_maps.append(m)
    return in_maps


def _run(Q, K, V, cache_seqlens, trace=False, trace_cores=None):
    cs = np.asarray(cache_seqlens).astype(np.int64)
    nblks = tuple(
        int(min((int(cs[b]) + BLK - 1) // BLK, SMAX // BLK)) for b in range(B)
    )
    nc = _build(nblks)
    in_maps = _shard_inputs(Q, K, V, cache_seqlens, nblks)
    res = bass_utils.run_bass_kernel_spmd(
        nc,
        in_maps,
        core_ids=list(range(NCORES)),
        trace=trace,
        trace_cores=trace_cores,
    )
    out = np.empty((B, SQ, H, DV), np.float32)
    for c in range(NCORES):
        for b in range(B):
            # stored as out^T [dv, q]; undo on host
            out[b, :, c * G : (c + 1) * G, :] = (
                res.results[c]["out"][b].T.reshape(SQ, G, DV).astype(np.float32)
            )
    return out, res


def kernel(Q, K, V, cache_seqlens):
    out, _ = _run(Q, K, V, cache_seqlens)
    return out
